# revision 101
# baseline (speedup 1.0000x reference)
"""Trainium2 Bass kernel for nn_Attention_49709951484392 (causal attention
block: LN1 -> QKV -> key smearing -> causal attention with learned ALiBi ->
out-proj -> LN2), sharded over 8 NeuronCores.

Sharding: core c handles batch c//4 and head-group c%4 (4 of 16 heads).
Out-projection partial sums are ReduceScatter'ed over each batch's 4-core
group (3 chunked collectives pipelined against compute); each core then runs
LN2 on its 512-row slice of the output.

v1 redesign vs baseline:
  - bf16 everywhere on the matmul paths (x, h, qT/kT, at, v, oT, wo): every
    PE row is 1 cycle (f32r pays 4x on <256-wide diagonal partials).
  - LN1 transposes moved off the PE onto the DMA xbar (dma_start_transpose,
    hT[p,c,j] = h[j, c*128+p]) - PE does only GEMMs.
  - smear as ts+scalar_tensor_tensor on DVE (bf16) instead of Pool adds.
  - attention software-pipelined: S(kbi+1) emitted before PV(kbi) so the PE
    stream never blocks on the ACT Exp; exp is the attention-phase bound.
  - 3 chunked ReduceScatters (rows 0:512, 512:1024, 1024:2048) so only the
    last (~28us) is exposed; LN2 emitted after attention so the in-order ACT
    stream is never blocked mid-attention.

Same augmented-matmul softmax trick as baseline: kT row 64 = 8.0, qT row 64 =
-M_i (overflow guard, cancels in softmax), exp bias = per-partition alibi,
PV aug column of ones produces the denominator in ops row 64.
"""
import sys

import numpy as np

sys.path.insert(0, "/opt/trn_rl_repo")

import concourse.bacc as bacc
import concourse.bass as bass
import concourse.mybir as mybir
import concourse.tile as tile
from concourse.bass_utils import run_bass_kernel_spmd

F32 = mybir.dt.float32
F32R = mybir.dt.float32r
BF16 = mybir.dt.bfloat16
AF = mybir.ActivationFunctionType
ALU = mybir.AluOpType
AX = mybir.AxisListType

HEADS = 16
DH = 64
DM = 1024
B, L = 2, 2048
EPS = 1e-5
NCORES = 8
HG = 4          # heads per core
FL = HG * DH    # local feature width (256)
QB = 1024       # query block
NK = L // 128   # key blocks of 128
NLT = L // 128  # l-tiles

_CACHE = {}
PHASE_MARKS = []


def _mark(name, nc):
    ids = []
    for k in nc.inst_map.keys():
        if isinstance(k, str) and k.startswith("I-"):
            try:
                ids.append(int(k.split("-")[1]))
            except ValueError:
                pass
    PHASE_MARKS.append((name, max(ids) if ids else 0))


def _build_program(trivial_ln2):
    nc = bacc.Bacc()
    xin = nc.declare_dram_parameter("xin", [L, DM], BF16, isOutput=False)
    wqk_d = nc.declare_dram_parameter("wqk", [128, 8, 2 * FL], BF16,
                                      isOutput=False)
    wv_d = nc.declare_dram_parameter("wv", [128, 8, FL], BF16, isOutput=False)
    wo_d = nc.declare_dram_parameter("wo", [128, 2, DM], BF16, isOutput=False)
    bqkr_d = nc.declare_dram_parameter("bqkr", [1, 2 * FL], BF16,
                                       isOutput=False)
    bvr_d = nc.declare_dram_parameter("bvr", [1, FL], BF16, isOutput=False)
    srep_d = nc.declare_dram_parameter("srep", [128, 2], F32, isOutput=False)
    omsrep_d = nc.declare_dram_parameter("omsrep", [128, 2], F32,
                                         isOutput=False)
    alibi_d = nc.declare_dram_parameter("alibi", [HG, 128, NK], F32,
                                        isOutput=False)
    aliq_d = nc.declare_dram_parameter("aliq", [2, 2, L], BF16, isOutput=False)
    if not trivial_ln2:
        ln2g_d = nc.declare_dram_parameter("ln2g", [DM], F32, isOutput=False)
        ln2b_d = nc.declare_dram_parameter("ln2b", [DM], F32, isOutput=False)
    else:
        ln2g_d = ln2b_d = None
    out_d = nc.declare_dram_parameter("out", [L // 4, DM], F32, isOutput=True)

    from contextlib import ExitStack
    with tile.TileContext(nc) as tc, ExitStack() as ctx:
        _emit(ctx, nc, tc, xin, wqk_d, wv_d, wo_d, bqkr_d, bvr_d,
              srep_d, omsrep_d, alibi_d, aliq_d, ln2g_d, ln2b_d, out_d,
              trivial_ln2)
    nc.compile()
    return nc


def _bcast_ap(handle, parts, free):
    ap = handle[:]
    return bass.AP(tensor=ap.tensor, offset=0, ap=[[0, parts], [1, free]])


def _emit(ctx, nc, tc, xin, wqk_d, wv_d, wo_d, bqkr_d, bvr_d,
          srep_d, omsrep_d, alibi_d, aliq_d, ln2g_d, ln2b_d, out_d,
          trivial_ln2):
    from contextlib import ExitStack

    consts = ctx.enter_context(tc.tile_pool(name="consts", bufs=1))
    persist = ctx.enter_context(tc.tile_pool(name="persist", bufs=1))
    dram = ctx.enter_context(tc.tile_pool(name="dram", bufs=1, space="DRAM"))

    eps_t = consts.tile([128, 1], F32)
    nc.vector.memset(eps_t, EPS)
    ones64_f = consts.tile([1, 64], F32)
    nc.vector.memset(ones64_f, 1.0)
    ones64_r = consts.tile([1, 64], F32R)
    nc.vector.tensor_copy(out=ones64_r, in_=ones64_f)
    ones512_b = consts.tile([1, 512], BF16)
    nc.vector.memset(ones512_b, 1.0)
    ones128_b = consts.tile([1, 128], BF16)
    nc.vector.memset(ones128_b, 1.0)
    bd_f = consts.tile([128, 2], F32)
    nc.vector.memset(bd_f, 0.0)
    nc.vector.memset(bd_f[0:64, 0:1], 1.0)
    nc.vector.memset(bd_f[64:128, 1:2], 1.0)
    bd_r = consts.tile([128, 2], F32R)
    nc.vector.tensor_copy(out=bd_r, in_=bd_f)
    from concourse.masks import make_identity
    ident_f = consts.tile([128, 128], F32)
    make_identity(nc, ident_f)
    ident_b = consts.tile([128, 128], BF16)
    nc.vector.tensor_copy(out=ident_b, in_=ident_f)

    # small runtime vectors on the scalar-engine HWDGE queue
    bqkr_t = consts.tile([1, 2 * FL], BF16)
    nc.scalar.dma_start(out=bqkr_t, in_=bqkr_d[:, :])
    bvr_t = consts.tile([1, FL], BF16)
    nc.scalar.dma_start(out=bvr_t, in_=bvr_d[:, :])
    s_t = consts.tile([128, 2], F32)
    nc.scalar.dma_start(out=s_t, in_=srep_d[:, :])
    oms_t = consts.tile([128, 2], F32)
    nc.scalar.dma_start(out=oms_t, in_=omsrep_d[:, :])
    aliq_t = [consts.tile([2, L], BF16, name=f"aliq{p}") for p in range(2)]
    for p in range(2):
        nc.scalar.dma_start(out=aliq_t[p], in_=aliq_d[p, :, :])
    alibi_t = [consts.tile([128, NK], F32, name=f"ali{h}") for h in range(HG)]
    for h in range(HG):
        nc.gpsimd.dma_start(out=alibi_t[h], in_=alibi_d[h, :, :])
    if not trivial_ln2:
        g2b_t = consts.tile([128, DM], F32)
        nc.gpsimd.dma_start(out=g2b_t, in_=_bcast_ap(ln2g_d, 128, DM))
        b2b_t = consts.tile([128, DM], F32)
        nc.gpsimd.dma_start(out=b2b_t, in_=_bcast_ap(ln2b_d, 128, DM))

    # persistent activation tiles
    hT = persist.tile([128, 8, L], BF16, name="hT")
    qT = [persist.tile([65, L], BF16, name=f"qT{h}") for h in range(HG)]
    kT = [persist.tile([65, L], BF16, name=f"kT{h}") for h in range(HG)]
    v_sb = persist.tile([128, NLT, HG, 65], F32R, name="v_sb")
    oT = [persist.tile([128, L], BF16, name=f"oT{m}") for m in range(2)]
    wqk8 = persist.tile([128, 8, 2 * FL], BF16, name="wqk8")
    wv8 = persist.tile([128, 8, FL], BF16, name="wv8")
    wo2 = persist.tile([128, 2, DM], BF16, name="wo2")
    qn_bf = [persist.tile([2, L], BF16, name=f"qn{p}") for p in range(2)]

    # kT aug row (8.0) + v aug col (1.0): emit early, fully hidden
    for h in range(HG):
        nc.gpsimd.memset(kT[h][64:65, :], 8.0)
    onesv_f = consts.tile([128, NLT * HG], F32)
    nc.vector.memset(onesv_f, 1.0)
    nc.vector.tensor_copy(
        out=v_sb[:, :, :, 64:65],
        in_=onesv_f.rearrange("p (a b c) -> p a b c", a=NLT, b=HG))

    ypart = dram.tile([L, DM], BF16, name="ypart")
    # RS chunks over q rows, in completion order: the expensive q-chunk
    # (rows 1024:2048) is processed FIRST so its collective overlaps the
    # cheap chunk's attention; the cheap chunk ships as one RS at the end.
    # Two RS (not more): each collective has ~15us fixed overhead.
    RS_ROWS = [(0, 1024), (1024, 1024)]
    yred = [dram.tile([rows // 4, DM], BF16, name=f"yred{i}")
            for i, (_, rows) in enumerate(RS_ROWS)]

    _mark('ph1', nc)
    # ---- Phase 1+2 fused: per group of 4 l-tiles, LN1 + DMA-xbar transpose,
    #      then the QK GEMM N-tile over those columns.  N-order [2,3,0,1]:
    #      the expensive q-chunk (rows 1024:2048) runs first in attention, so
    #      its qT columns + M rows (n2,n3) and first kT blocks (n0) are
    #      produced as early as possible. ----
    kbcol = {}
    kc0sav = {}
    with ExitStack() as ph1:
        x4p = ph1.enter_context(tc.tile_pool(name="x4p", bufs=2))
        hp = ph1.enter_context(tc.tile_pool(name="hp", bufs=4))
        stp = ph1.enter_context(tc.tile_pool(name="stp", bufs=6))
        psq = ph1.enter_context(tc.tile_pool(name="psq", bufs=3, space="PSUM"))
        psT = ph1.enter_context(tc.tile_pool(name="psT", bufs=2, space="PSUM"))
        psn2 = ph1.enter_context(tc.tile_pool(name="psn2", bufs=1,
                                              space="PSUM"))
        sqp = ph1.enter_context(tc.tile_pool(name="sqp", bufs=2))
        ktp = ph1.enter_context(tc.tile_pool(name="ktp", bufs=2))
        mtp = ph1.enter_context(tc.tile_pool(name="mtp", bufs=2))

        psv = ph1.enter_context(tc.tile_pool(name="psv", bufs=2, space="PSUM"))
        xr = xin.rearrange("(i j p) d -> i p j d", j=4, p=128)
        nc.sync.dma_start(out=wqk8[:, :, 0:256], in_=wqk_d[:, :, 0:256])
        nc.sync.dma_start(out=wqk8[:, :, 256:512], in_=wqk_d[:, :, 256:512])
        nc.sync.dma_start(out=wv8, in_=wv_d[:, :, :])
        N_ORDER = (0, 1, 2, 3)
        for ni, n in enumerate(N_ORDER):
            x4 = x4p.tile([128, 4, DM], BF16, name="x4", tag="x4")
            if ni == 0:
                nc.sync.dma_start(out=x4[:, 0:2, :], in_=xr[n][:, 0:2, :])
                nc.sync.dma_start(out=x4[:, 2:4, :], in_=xr[n][:, 2:4, :])
            else:
                nc.sync.dma_start(out=x4, in_=xr[n])
            if ni == 2:
                nc.sync.dma_start(out=wo2, in_=wo_d[:, :, :])
            for j4 in range(4):
                lt = 4 * n + j4
                x_t = x4[:, j4, :]
                st = stp.tile([128, 2, 6], F32)
                nc.vector.bn_stats(out=st[:, 0, :], in_=x_t[:, 0:512])
                nc.vector.bn_stats(out=st[:, 1, :], in_=x_t[:, 512:1024])
                mv = stp.tile([128, 2], F32)
                nc.vector.bn_aggr(out=mv, in_=st)
                rstd = stp.tile([128, 1], F32)
                nc.scalar.activation(out=rstd, in_=mv[:, 1:2], func=AF.Sqrt,
                                     bias=eps_t, scale=1.0)
                nc.vector.reciprocal(out=rstd, in_=rstd)
                h_bf = hp.tile([128, DM], BF16, name="h_bf", tag="h_bf")
                with nc.allow_low_precision(reason="bf16 activations"):
                    nc.vector.tensor_scalar(out=h_bf, in0=x_t,
                                            scalar1=mv[:, 0:1], scalar2=rstd,
                                            op0=ALU.subtract, op1=ALU.mult)
                # PE transpose (bf16, 1 cyc/row) into PSUM, then evacuate
                for g in range(2):
                    pst = psT.tile([128, 512], BF16, name="pst", tag="pst")
                    for jj in range(4):
                        dc = 4 * g + jj
                        nc.tensor.transpose(pst[:, jj * 128:(jj + 1) * 128],
                                            h_bf[:, dc * 128:(dc + 1) * 128],
                                            ident_b)
                    ceng = nc.scalar.copy if g == 0 else \
                        (lambda out, in_: nc.vector.tensor_copy(out=out,
                                                                in_=in_))
                    with nc.allow_low_precision(reason="bf16 activations"):
                        ceng(out=hT[:, 4 * g:4 * g + 4,
                                    lt * 128:(lt + 1) * 128],
                             in_=pst.rearrange("p (a b) -> p a b", a=4))
            # QK GEMM for this N-tile (columns n*512 .. n*512+512).
            # q pairs first for n2/n3 (feeds ch1's M rows), k pairs first
            # for n0/n1 (feeds ch1's first S blocks).
            nsl = slice(n * 512, (n + 1) * 512)
            m_order = (0, 1, 2, 3) if n < 2 else (2, 3, 0, 1)
            for m in m_order:       # 0,1: q head-pairs; 2,3: k head-pairs
                pair = m % 2
                is_q = m < 2
                ps = psq.tile([128, 512], F32, name="psqk", tag="psqk")
                nc.tensor.matmul(ps, bqkr_t[:, m * 128:(m + 1) * 128],
                                 ones512_b, start=True, stop=False)
                for kc in range(8):
                    nc.tensor.matmul(
                        ps, wqk8[:, kc, m * 128:(m + 1) * 128],
                        hT[:, kc, nsl],
                        start=False, stop=(kc == 7))
                if is_q:
                    # row-norm statistics for the softmax guard: sq = x^2 on
                    # ACT, then a blockdiag column-sum matmul -> head norms
                    sq_t = sqp.tile([128, 512], F32R, name="sq", tag="sq")
                    nc.scalar.activation(out=sq_t, in_=ps, func=AF.Square,
                                         bias=0.0, scale=1.0)
                    pn2 = psn2.tile([2, 512], F32, name="pn2", tag="pn2")
                    nc.tensor.matmul(pn2, bd_r, sq_t, start=True, stop=True)
                    nc.scalar.copy(out=qn_bf[pair][:, nsl], in_=pn2)
                    for hh in range(2):
                        h = pair * 2 + hh
                        rows = slice(hh * 64, (hh + 1) * 64)
                        nc.scalar.copy(out=qT[h][0:64, nsl],
                                       in_=ps[rows, :])
                    continue
                # k tiles: evacuate PSUM once, then smear from SBUF (bf16):
                # kT_j = (1-s)*k_j + s*k_{j-1}
                kbf = ktp.tile([128, 512], BF16, name="kbf", tag="kbf")
                with nc.allow_low_precision(reason="bf16 activations"):
                    nc.vector.tensor_copy(out=kbf, in_=ps)
                tmp = ktp.tile([128, 512], BF16, name="ktmp", tag="ktmp")
                with nc.allow_low_precision(reason="bf16 activations"):
                    nc.vector.tensor_scalar(
                        out=tmp, in0=kbf, scalar1=s_t[:, pair:pair + 1],
                        scalar2=None, op0=ALU.mult)
                c0 = n * 512
                for hh in range(2):
                    h = pair * 2 + hh
                    rows = slice(hh * 64, (hh + 1) * 64)
                    with nc.allow_low_precision(reason="bf16 activations"):
                        nc.vector.scalar_tensor_tensor(
                            out=kT[h][0:64, c0 + 1:c0 + 512],
                            in0=kbf[rows, 1:512],
                            scalar=oms_t[rows, pair:pair + 1],
                            in1=tmp[rows, 0:511],
                            op0=ALU.mult, op1=ALU.add)
                        if n == 0:
                            nc.vector.tensor_scalar(
                                out=kT[h][0:64, 0:1], in0=kbf[rows, 0:1],
                                scalar1=oms_t[rows, pair:pair + 1],
                                scalar2=None, op0=ALU.mult)
                        else:
                            nc.vector.scalar_tensor_tensor(
                                out=kT[h][0:64, c0:c0 + 1],
                                in0=kbf[rows, 0:1],
                                scalar=oms_t[rows, pair:pair + 1],
                                in1=kbcol[h][rows, :],
                                op0=ALU.mult, op1=ALU.add)
                    if n < 3:
                        bc = ktp.tile([128, 1], BF16, name=f"kb{h}",
                                      tag=f"kb{h}", bufs=2)
                        nc.gpsimd.tensor_copy(out=bc[rows, :],
                                              in_=tmp[rows, 511:512])
                        kbcol[h] = bc
            # -M rows of qT: -M = -qn/16 - (relu(slope)*i + kbound/16); the
            # k-norm bound is folded into aliq host-side, so M needs only
            # this half's q norms -> emitted per half, as early as possible.
            if ni in (1, 3):
                hsl = slice(0, 1024) if ni == 1 else slice(1024, 2048)
                for pair in range(2):
                    stag = mtp.tile([2, 1024], BF16, name="stag", tag="stag")
                    with nc.allow_low_precision(reason="cancels in softmax"):
                        nc.vector.tensor_scalar(
                            out=stag, in0=qn_bf[pair][:, hsl],
                            scalar1=-1.0 / 16.0, scalar2=None, op0=ALU.mult)
                        nc.vector.tensor_tensor(
                            out=stag, in0=stag, in1=aliq_t[pair][:, hsl],
                            op=ALU.subtract)
                    for hh in range(2):
                        nc.sync.dma_start(
                            out=qT[pair * 2 + hh][64:65, hsl],
                            in_=stag[hh:hh + 1, :])


        # V GEMM (baseline-proven shape: natural lt order, bias last)
        for lt in range(NLT):
            ps = psv.tile([128, FL], F32, name="psv", tag="psv")
            for kc in range(8):
                nc.tensor.matmul(
                    ps, hT[:, kc, lt * 128:(lt + 1) * 128],
                    wv8[:, kc, :], start=(kc == 0), stop=False)
            nc.tensor.matmul(ps, ones128_b, bvr_t, start=False, stop=True)
            nc.scalar.copy(
                out=v_sb[:, lt, :, 0:64],
                in_=ps.rearrange("p (a b) -> p a b", a=HG))

    _mark('ph4', nc)
    # ---- Attention + out-proj + chunked ReduceScatter ----
    NCH = L // QB  # chunks (2)
    with ExitStack() as s3:
        psS = s3.enter_context(tc.tile_pool(name="psS", bufs=2, space="PSUM"))
        psO = s3.enter_context(tc.tile_pool(name="psO", bufs=1, space="PSUM"))
        psY = s3.enter_context(tc.tile_pool(name="psY", bufs=2, space="PSUM"))
        atp = s3.enter_context(tc.tile_pool(name="atp", bufs=6))
        nrm = s3.enter_context(tc.tile_pool(name="nrm", bufs=2))
        ysp = s3.enter_context(tc.tile_pool(name="ysp", bufs=4))
        ln2p = s3.enter_context(tc.tile_pool(name="ln2p", bufs=4))

        import os as _os
        dbg3 = _os.environ.get("KDUMP") == "3"
        if dbg3:
            dbgp = s3.enter_context(tc.tile_pool(name="dbgp", bufs=1))
            at_d0 = dbgp.tile([128, QB], BF16, name="at_d0")
            at_d8 = dbgp.tile([128, QB], BF16, name="at_d8")
            osb_d = dbgp.tile([65, QB], F32, name="osb_d")
        for qb in (0, 1):
            qlo = qb * QB
            qsl = slice(qlo, qlo + QB)
            for h in (1, 3, 0, 2):
                ops = psO.tile([65, QB], F32, name="ops", tag="ops")
                nkb = (qlo + QB) // 128
                last_kbi = [(qlo + 512) // 128 - 1, nkb - 1]
                pend = None

                def finish(kbi, sps):
                    kb = kbi * 128
                    off = max(0, kb - qlo)
                    at = atp.tile([128, QB], F32R, name="at", tag="at")
                    with nc.allow_low_precision(reason="bf16 probs"):
                        nc.scalar.activation(out=at[:, off:QB],
                                             in_=sps[:, off:QB], func=AF.Exp,
                                             bias=alibi_t[h][:, kbi:kbi + 1],
                                             scale=0.125)
                    if kb >= qlo:
                        # fill-replacement masks the +inf above-diagonal exps
                        nc.gpsimd.affine_select(
                            out=at[:, off:off + 128],
                            in_=at[:, off:off + 128],
                            compare_op=ALU.is_ge, fill=0.0, base=0,
                            channel_multiplier=-1, pattern=[[1, 128]])
                    if dbg3 and qb == 1 and h == 0 and kbi in (0, 8):
                        nc.vector.tensor_copy(
                            out=(at_d0 if kbi == 0 else at_d8)[:, off:QB],
                            in_=at[:, off:QB])
                    for half in range(2):
                        r0, r1 = max(off, half * 512), (half + 1) * 512
                        if r0 >= r1:
                            continue
                        nc.tensor.matmul(ops[:, r0:r1], v_sb[:, kbi, h, :],
                                         at[:, r0:r1],
                                         start=(kbi == 0),
                                         stop=(kbi == last_kbi[half]))

                for kbi in range(nkb):
                    kb = kbi * 128
                    off = max(0, kb - qlo)
                    sps = psS.tile([128, QB], F32, name="sps", tag="sps")
                    for half in range(2):
                        r0, r1 = max(off, half * 512), (half + 1) * 512
                        if r0 >= r1:
                            continue
                        nc.tensor.matmul(sps[:, r0:r1], kT[h][:, kb:kb + 128],
                                         qT[h][:, qlo + r0:qlo + r1],
                                         start=True, stop=True)
                    if pend is not None:
                        finish(*pend)
                    pend = (kbi, sps)
                finish(*pend)

                # evacuate ops PSUM->SBUF immediately so the psO slot frees
                # for the next head's PV; normalize then runs off-critical.
                osb = nrm.tile([65, QB], F32, name="osb", tag="osb")
                nc.vector.tensor_copy(out=osb, in_=ops)
                if dbg3 and qb == 1 and h == 0:
                    nc.vector.tensor_copy(out=osb_d, in_=osb)
                dr = nrm.tile([1, QB], F32R, name="drr", tag="drr")
                with nc.allow_low_precision(reason="f32r is f32 bits"):
                    nc.vector.reciprocal(out=dr, in_=osb[64:65, :])
                if h % 2 == 1:
                    ostg = nrm.tile([64, QB], BF16, name="ostg", tag="ostg")
                for half in range(2):
                    hs = slice(half * 512, (half + 1) * 512)
                    bps = psY.tile([64, 512], F32, name="bps", tag="psy")
                    nc.tensor.matmul(bps, ones64_r, dr[:, hs],
                                     start=True, stop=True)
                    with nc.allow_low_precision(reason="bf16 activations"):
                        if h % 2 == 0:
                            nc.vector.tensor_mul(
                                out=oT[h // 2][0:64, qlo + half * 512:
                                               qlo + (half + 1) * 512],
                                in0=osb[0:64, hs], in1=bps)
                        else:
                            nc.vector.tensor_mul(out=ostg[:, hs],
                                                 in0=osb[0:64, hs], in1=bps)
                if h % 2 == 1:
                    nc.sync.dma_start(out=oT[h // 2][64:128, qsl], in_=ostg)

            # out-proj for this q-chunk (separate psY pool so its PSUM slots
            # never gate the next chunk's S matmuls)
            for j in range(QB // 128):
                lt = qb * (QB // 128) + j
                ysb = ysp.tile([128, DM], BF16, name="ysb", tag="ysb")
                for nn in range(2):
                    psy = psY.tile([128, 512], F32, name="psy", tag="psy")
                    for kc in range(2):
                        nc.tensor.matmul(psy,
                                         oT[kc][:, lt * 128:(lt + 1) * 128],
                                         wo2[:, kc, nn * 512:(nn + 1) * 512],
                                         start=(kc == 0), stop=(kc == 1))
                    with nc.allow_low_precision(reason="bf16 partials"):
                        nc.vector.tensor_copy(
                            out=ysb[:, nn * 512:(nn + 1) * 512], in_=psy)
                eng = nc.sync if j % 2 == 0 else nc.scalar
                eng.dma_start(out=ypart[lt * 128:(lt + 1) * 128, :],
                              in_=ysb)
                for ri, (r0, rows) in enumerate(RS_ROWS):
                    if (lt + 1) * 128 == r0 + rows:
                        nc.gpsimd.collective_compute(
                            "ReduceScatter", ALU.add,
                            replica_groups=[[0, 1, 2, 3], [4, 5, 6, 7]],
                            ins=[ypart[r0:r0 + rows, :]],
                            outs=[yred[ri][:, :]])

        _mark('ph7', nc)
        if dbg3:
            nc.gpsimd.dma_start(out=out_d[0:128, :], in_=at_d0)
            nc.gpsimd.dma_start(out=out_d[128:256, :], in_=at_d8)
            nc.gpsimd.dma_start(out=out_d[256:321, :], in_=osb_d)
            return
        if _os.environ.get("KDUMP") == "4":
            # dump v_sb: [128, lt, head0+head1, 65] for lt 0..7
            for lt in range(8):
                nc.gpsimd.dma_start(
                    out=out_d[lt * 64:(lt + 1) * 64, 0:130],
                    in_=v_sb[0:64, lt, 0:2, :])
                nc.gpsimd.dma_start(
                    out=out_d[512 - (lt + 1) * 64:512 - lt * 64, 0:130],
                    in_=v_sb[0:64, 8 + lt, 0:2, :])
            return
        if _os.environ.get("KDUMP") == "2":
            # debug: dump qT[0] and kT[0]
            for half in range(2):
                nc.gpsimd.dma_start(
                    out=out_d[half * 65:(half + 1) * 65, :],
                    in_=qT[0][:, half * 1024:(half + 1) * 1024])
                nc.gpsimd.dma_start(
                    out=out_d[130 + half * 65:130 + (half + 1) * 65, :],
                    in_=kT[0][:, half * 1024:(half + 1) * 1024])
                nc.gpsimd.dma_start(
                    out=out_d[260 + half * 65:260 + (half + 1) * 65, :],
                    in_=qT[2][:, half * 1024:(half + 1) * 1024])
            return
        if _os.environ.get("KDUMP"):
            # debug: dump oT (normalized attention outputs) instead of LN2
            for m in range(2):
                for half in range(2):
                    nc.gpsimd.dma_start(
                        out=out_d[(2 * m + half) * 128:
                                  (2 * m + half + 1) * 128, :],
                        in_=oT[m][:, half * 1024:(half + 1) * 1024])
            return
        # ---- LN2: pushed to the end of the schedule (tile_wait_until) so
        #      its RS-gated waits never block engine queues mid-attention ----
        orow = 0
        for sc, (_, rows) in enumerate(RS_ROWS):
            s3.enter_context(tc.tile_wait_until(1.0 + 0.2 * sc))
            for ti in range(rows // 512):
                y_t = ln2p.tile([128, DM], BF16, name="y2t", tag="y2t")
                nc.gpsimd.dma_start(out=y_t,
                                    in_=yred[sc][ti * 128:(ti + 1) * 128, :])
                st = ln2p.tile([128, 2, 6], F32, name="st2", tag="st2")
                nc.vector.bn_stats(out=st[:, 0, :], in_=y_t[:, 0:512])
                nc.vector.bn_stats(out=st[:, 1, :], in_=y_t[:, 512:1024])
                mv = ln2p.tile([128, 2], F32, name="mv2", tag="mv2")
                nc.vector.bn_aggr(out=mv, in_=st)
                rstd = ln2p.tile([128, 1], F32, name="rstd2", tag="rstd2")
                nc.scalar.activation(out=rstd, in_=mv[:, 1:2], func=AF.Sqrt,
                                     bias=eps_t, scale=1.0)
                nc.vector.reciprocal(out=rstd, in_=rstd)
                o_t = ln2p.tile([128, DM], F32, name="o2t", tag="o2t")
                nc.vector.tensor_scalar(out=o_t, in0=y_t, scalar1=mv[:, 0:1],
                                        scalar2=rstd, op0=ALU.subtract,
                                        op1=ALU.mult)
                if not trivial_ln2:
                    nc.vector.tensor_tensor(out=o_t, in0=o_t, in1=g2b_t,
                                            op=ALU.mult)
                    nc.vector.tensor_tensor(out=o_t, in0=o_t, in1=b2b_t,
                                            op=ALU.add)
                # out writes on the ACT queue: keeps the gpsimd queue free to
                # issue the next tile's y_t load immediately
                nc.scalar.dma_start(out=out_d[orow:orow + 128, :], in_=o_t)
                orow += 128


def _prep_inputs(x, ln1_g, ln1_b, in_w, out_w, ln2_g, ln2_b, slopes, smear):
    """Slice/transpose per-core views of the weights (host-side marshaling)."""
    import ml_dtypes
    bf = ml_dtypes.bfloat16
    x = np.asarray(x, np.float32)
    in_w = np.asarray(in_w, np.float32)
    out_w = np.asarray(out_w, np.float32)
    ln1_g = np.asarray(ln1_g, np.float32)
    ln1_b = np.asarray(ln1_b, np.float32)
    slopes = np.asarray(slopes, np.float32)
    smear = np.asarray(smear, np.float32)
    w_eff = in_w * ln1_g[None, :]
    qkvb = in_w @ ln1_b
    sig = 1.0 / (1.0 + np.exp(-smear))
    # per-head spectral bound on |k_j|^2: |k| <= smax(W_k_h)*|h| + |b_k_h|
    # with |h| < 32 exactly (LN output, g/b folded into W).  Folded into
    # aliq as kbound/16 so the softmax guard M needs no runtime k-statistics.
    kbound = np.empty(HEADS, np.float32)
    for h in range(HEADS):
        wk_h = w_eff[DM + 64 * h:DM + 64 * (h + 1)].astype(np.float64)
        smax = np.linalg.svd(wk_h, compute_uv=False)[0]
        bk = np.linalg.norm(qkvb[DM + 64 * h:DM + 64 * (h + 1)])
        kbound[h] = (1.01 * 32.2 * smax + bk) ** 2
    in_maps = []
    for c in range(NCORES):
        b, hg = c // 4, c % 4
        f0 = FL * hg
        wq = w_eff[f0:f0 + FL]
        wk = w_eff[DM + f0:DM + f0 + FL]
        wv = w_eff[2 * DM + f0:2 * DM + f0 + FL]
        sl4 = slopes[4 * hg:4 * hg + 4]
        p = np.arange(128, dtype=np.float32)
        kbv = np.arange(NK, dtype=np.float32) * 128.0
        alibi = sl4[:, None, None] * (kbv[None, None, :] + p[None, :, None])
        aliq = np.maximum(sl4, 0.0)[:, None] \
            * np.arange(L, dtype=np.float32)[None, :] \
            + kbound[4 * hg:4 * hg + 4, None] / 16.0
        wqk = np.concatenate([wq, wk], 0).T        # [DM, 512]
        wvt = wv.T                                 # [DM, 256]
        wot = out_w[:, f0:f0 + FL].T               # [256, DM]
        sg = np.repeat(sig[4 * hg:4 * hg + 4].reshape(2, 2), 64, axis=1)
        in_maps.append({
            "xin": np.ascontiguousarray(x[b]).astype(bf),
            "wqk": np.ascontiguousarray(
                wqk.reshape(8, 128, 2 * FL).transpose(1, 0, 2)).astype(bf),
            "wv": np.ascontiguousarray(
                wvt.reshape(8, 128, FL).transpose(1, 0, 2)).astype(bf),
            "wo": np.ascontiguousarray(
                wot.reshape(2, 128, DM).transpose(1, 0, 2)).astype(bf),
            "bqkr": np.ascontiguousarray(
                np.concatenate([qkvb[f0:f0 + FL],
                                qkvb[DM + f0:DM + f0 + FL]])[None, :]
            ).astype(bf),
            "bvr": np.ascontiguousarray(
                qkvb[2 * DM + f0:2 * DM + f0 + FL][None, :]).astype(bf),
            "srep": np.ascontiguousarray(sg.T.astype(np.float32)),
            "omsrep": np.ascontiguousarray((1.0 - sg.T).astype(np.float32)),
            "alibi": np.ascontiguousarray(alibi.astype(np.float32)),
            "aliq": np.ascontiguousarray(
                aliq.reshape(2, 2, L)).astype(bf),
            "ln2g": np.asarray(ln2_g, np.float32),
            "ln2b": np.asarray(ln2_b, np.float32),
        })
    return in_maps


def kernel(**inputs):
    trivial_ln2 = (np.allclose(np.asarray(inputs["ln2_g"]), 1.0)
                   and np.allclose(np.asarray(inputs["ln2_b"]), 0.0))
    key = ("nc", trivial_ln2)
    if key not in _CACHE:
        _CACHE[key] = _build_program(trivial_ln2)
        _CACHE["nc"] = _CACHE[key]
    nc = _CACHE[key]
    in_maps = _prep_inputs(**inputs)
    if trivial_ln2:
        for m in in_maps:
            m.pop("ln2g")
            m.pop("ln2b")
    res = run_bass_kernel_spmd(nc, in_maps, core_ids=list(range(NCORES)))
    out = np.empty((B, L, DM), np.float32)
    for c in range(NCORES):
        b, hg = c // 4, c % 4
        r = res.results[c]["out"]
        rs_rows = [(0, 1024), (1024, 1024)]
        orow = 0
        for r0, rows in rs_rows:
            p = rows // 4
            out[b, r0 + hg * p:r0 + (hg + 1) * p, :] = r[orow:orow + p]
            orow += p
    return out
